# revision 1
# baseline (speedup 1.0000x reference)
"""GCN+MLP Trainium2 kernel: 8-core SPMD, NKI sparse aggregation + XLA dense.

Model (see harness reference): embed -> 2x ChebConv(K=2) -> mean-pool -> MLP
-> broadcast. N=65536 nodes, 1M random edges (uniform), EMB=128.

Distribution: nodes dst-sharded 8192/core (= 4 complete batches, so the mean
pool is local). Edges are routed on host to the core owning their dst and
sorted by dst. Aggregation per core, per dst-group of 128 nodes:
  - indirect row DMA (vector DGE) gathers 128 source rows per chunk,
  - a one-hot matrix (iota==dst_local)*wgt built on the vector engine,
  - TensorEngine matmul accumulates chunk messages into PSUM: txT += G.T @ OH.
ChebConv linear algebra identities move all per-edge scaling into the one-hot
(wgt = -dinv[src]*dinv[dst]) and the embed matmul *after* aggregation for
conv0 (aggregate raw x, then multiply by embed_W). The h1 table is
all-gathered (bf16, 2MB/core) between conv layers; dense matmuls, pooling and
the MLP run in XLA.
"""

import numpy as np

import jax
import jax.numpy as jnp
from jax.sharding import Mesh, PartitionSpec as P
from jax.experimental.shard_map import shard_map

import neuronxcc.nki as nki
import neuronxcc.nki.isa as nisa
import neuronxcc.nki.language as nl

B, E, D = 32, 2048, 64
EMB, HID, PRED, NPRED = 128, 64, 3, 12
N = B * E
NCORES = 8
NPC = N // NCORES          # 8192 nodes/core
BPC = B // NCORES          # 4 batches/core

NGRP = 64                  # dst groups of 128 per core
DGRP = NPC // NGRP         # 128
NCH = 18                   # chunks of 128 edge slots per group (mean 16)
NCHK = NGRP * NCH          # 1152 chunks per core per layer

_CACHE = {}


def _agg_kernel_factory(PF, bf16):
    """Aggregation kernel: table [N, PF] -> txT [PF, NPC] f32."""
    mdt = nl.bfloat16 if bf16 else nl.float32

    @nki.jit
    def agg_kernel(table, idxs, dstw, iota):
        # table: [N, PF] (bf16|f32) node-major gather source in HBM
        # idxs:  [128, NCHK] int32   (chunk c -> column c)
        # dstw:  [2, 128, NCHK] (f32) [dst_local | wgt]
        # iota:  [128, DGRP] (mdt)  iota[e, d] = d
        out = nl.ndarray((PF, NPC), dtype=nl.float32, buffer=nl.shared_hbm)
        ix = nl.load(idxs)                          # [128, NCHK]
        dl = nl.load(dstw[0])                       # [128, NCHK] f32
        wg = nl.load(dstw[1])                       # [128, NCHK] f32
        io_t = nl.load(iota)                        # [128, DGRP]
        i_p = nl.arange(128)[:, None]
        i_f = nl.arange(PF)[None, :]
        for g in range(NGRP):
            ps0 = nl.zeros((PF, DGRP), dtype=nl.float32, buffer=nl.psum)
            ps1 = nl.zeros((PF, DGRP), dtype=nl.float32, buffer=nl.psum)
            tmp = nl.ndarray((nl.par_dim(128), NCH, PF), dtype=mdt,
                             buffer=nl.sbuf)
            for ch in range(NCH):
                c = g * NCH + ch
                nisa.dma_copy(dst=tmp[:, ch], src=table[ix[i_p, c], i_f],
                              dge_mode=nisa.dge_mode.hwdge)
            for ch in range(NCH):
                c = g * NCH + ch
                oh = nisa.tensor_scalar(
                    io_t, op0=nl.equal, operand0=dl[:, c:c + 1],
                    op1=nl.multiply, operand1=wg[:, c:c + 1],
                    dtype=mdt)
                if ch % 2 == 0:
                    ps0 += nisa.nc_matmul(tmp[:, ch], oh)
                else:
                    ps1 += nisa.nc_matmul(tmp[:, ch], oh)
            sl = slice(g * DGRP, (g + 1) * DGRP)
            out_sb = nl.add(ps0, ps1)
            nl.store(out[:, sl], out_sb)
        return out

    return agg_kernel


def _prep_edges(src, dst, dinv):
    """Route edges to cores, sort by dst, chunk per 128-dst groups."""
    idx_all = np.zeros((NCORES, 128, NCHK), np.int32)
    dstw_all = np.zeros((NCORES, 2, 128, NCHK), np.float32)
    wgt = (-dinv[src] * dinv[dst]).astype(np.float32)
    core = dst >> 13
    order = np.argsort(core * np.int64(N) + dst, kind="stable")
    src_s, dst_s, wgt_s = src[order], dst[order], wgt[order]
    cstart = np.searchsorted(core[order], np.arange(NCORES + 1))
    for c in range(NCORES):
        s_c = src_s[cstart[c]:cstart[c + 1]]
        d_c = dst_s[cstart[c]:cstart[c + 1]] - c * NPC
        w_c = wgt_s[cstart[c]:cstart[c + 1]]
        grp = d_c >> 7
        gstart = np.searchsorted(grp, np.arange(NGRP + 1))
        for g in range(NGRP):
            lo, hi = gstart[g], gstart[g + 1]
            n = hi - lo
            if n > NCH * 128:
                raise RuntimeError(f"group overflow {n} > {NCH * 128}")
            # slot j of group -> chunk j//128, partition j%128
            sl = np.arange(n)
            ch, pt = g * NCH + sl // 128, sl % 128
            idx_all[c, pt, ch] = s_c[lo:hi]
            dstw_all[c, 0, pt, ch] = (d_c[lo:hi] & 127).astype(np.float32)
            dstw_all[c, 1, pt, ch] = w_c[lo:hi]
            # padding slots keep wgt=0 -> zero contribution
    return idx_all, dstw_all


def kernel(**inputs):
    x = np.asarray(inputs["x"], np.float32)
    edge_index = np.asarray(inputs["edge_index"])
    W_e = np.asarray(inputs["embed_W"], np.float32)
    b_e = np.asarray(inputs["embed_b"], np.float32)
    W00 = np.asarray(inputs["conv0_W0"], np.float32)
    W01 = np.asarray(inputs["conv0_W1"], np.float32)
    b0 = np.asarray(inputs["conv0_b"], np.float32)
    W10 = np.asarray(inputs["conv1_W0"], np.float32)
    W11 = np.asarray(inputs["conv1_W1"], np.float32)
    b1 = np.asarray(inputs["conv1_b"], np.float32)
    mW1 = np.asarray(inputs["mlp_W1"], np.float32)
    mb1 = np.asarray(inputs["mlp_b1"], np.float32)
    mW2 = np.asarray(inputs["mlp_W2"], np.float32)
    mb2 = np.asarray(inputs["mlp_b2"], np.float32)

    src = np.asarray(edge_index[0], np.int64).astype(np.int32)
    dst = np.asarray(edge_index[1], np.int64).astype(np.int32)
    deg = np.bincount(src, minlength=N).astype(np.float32)
    dinv = np.where(deg > 0, 1.0 / np.sqrt(np.maximum(deg, 1e-12)), 0.0).astype(np.float32)

    idx_all, dstw_all = _prep_edges(src, dst, dinv)

    # rank-1 term for the embed bias routed through aggregation:
    # tx0 = Agg(x) @ We + sw * b_e, with sw[d] = sum of in-edge wgts of d
    sw = np.zeros(N, np.float32)
    np.add.at(sw, dst, (-dinv[src] * dinv[dst]))
    sw_sh = sw.reshape(NCORES, NPC)
    xT_sh = x.reshape(NCORES, NPC, D)
    iota = np.tile(np.arange(DGRP, dtype=np.float32), (128, 1))

    agg64 = _CACHE.setdefault("agg64", _agg_kernel_factory(D, True))
    agg128 = _CACHE.setdefault("agg128", _agg_kernel_factory(EMB, True))

    devices = jax.devices()[:NCORES]
    mesh = Mesh(np.array(devices), ("x",))

    consts = dict(
        W_e=jnp.asarray(W_e), b_e=jnp.asarray(b_e),
        W00=jnp.asarray(W00), W01=jnp.asarray(W01), b0=jnp.asarray(b0),
        W10=jnp.asarray(W10), W11=jnp.asarray(W11), b1=jnp.asarray(b1),
        mW1=jnp.asarray(mW1), mb1=jnp.asarray(mb1),
        mW2=jnp.asarray(mW2), mb2=jnp.asarray(mb2),
        xtab=jnp.asarray(x.reshape(N, D), jnp.bfloat16),
        iota32=jnp.asarray(iota, jnp.float32),
        iota16=jnp.asarray(iota, jnp.bfloat16),
    )
    const_names = list(consts.keys())
    const_vals = [consts[k] for k in const_names]

    def shard_fn(idx, dstw, xc, dinvc, swc, *cv):
        cst = dict(zip(const_names, cv))
        idx0, dstw0 = idx[0], dstw[0]
        xc, dinvc, swc = xc[0], dinvc[0], swc[0]
        # conv0: aggregate raw x rows, then embed the aggregate
        aggx = agg64(cst["xtab"], idx0, dstw0, cst["iota16"])   # [64, 8192] f32
        h0T = (xc @ cst["W_e"] + cst["b_e"]).T                   # [128, 8192]
        tx0T = cst["W_e"].T @ aggx + cst["b_e"][:, None] * swc[None, :]
        h1T = jax.nn.relu(cst["W00"].T @ h0T + cst["W01"].T @ tx0T + cst["b0"][:, None])
        u1 = h1T.T.astype(jnp.bfloat16)                          # [8192, 128] node-major
        u1full = jax.lax.all_gather(u1, "x", axis=0, tiled=True)  # [65536, 128]
        agg1 = agg128(u1full, idx0, dstw0, cst["iota16"])        # [128, 8192] f32
        h2T = jax.nn.relu(cst["W10"].T @ h1T + cst["W11"].T @ agg1 + cst["b1"][:, None])
        hm = h2T.reshape(EMB, BPC, E).mean(axis=2)               # [128, 4]
        z = jax.nn.relu(cst["mW1"].T @ hm + cst["mb1"][:, None])
        o = cst["mW2"].T @ z + cst["mb2"][:, None]               # [3, 4]
        return o.T[None]                                         # [1, 4, 3]

    fn = shard_map(
        shard_fn, mesh=mesh,
        in_specs=(P("x"), P("x"), P("x"), P("x"), P("x"))
                 + tuple(P() for _ in const_vals),
        out_specs=P("x"),
        check_rep=False,
    )
    jfn = jax.jit(fn)
    out = jfn(jnp.asarray(idx_all), jnp.asarray(dstw_all),
              jnp.asarray(xT_sh), jnp.asarray(dinv_sh := dinv.reshape(NCORES, NPC)),
              jnp.asarray(sw_sh), *const_vals)
    out1 = np.asarray(out, np.float32).reshape(B, PRED)
    full = np.broadcast_to(out1[:, None, None, :], (B, NPRED, E, PRED))
    return np.ascontiguousarray(full, dtype=np.float32)


kernel._jit_holder = _CACHE



# revision 20
# speedup vs baseline: 18.4340x; 18.4340x over previous
"""GCN+MLP Trainium2 kernel: single Bass/Tile NEFF over 8 cores.

Model (reference): h0 = x@We + be; two ChebConv(K=2) layers
  h_{l+1} = relu(h_l @ W0 + (S @ h_l) @ W1 + b),  S = scatter(wgt), wgt =
  -dinv[src]*dinv[dst]; then per-batch mean pool -> 2-layer MLP -> broadcast.

Distribution: dst-sharded, 8192 nodes (4 batches) per core.  Edges are routed
on host to the dst owner, sorted by (dst_group/128, src_half) and chunked into
128-edge matmul chunks.  Per chunk the kernel builds a weighted one-hot
oh[slot, dst_local] = (iota==dst)*wgt on the vector engine and accumulates
  psum[feat, dst] += gathered_rows[slot, feat]^T @ oh
on the tensor engine.  Layer-1 "gather" is done on host (x rows in edge
order, streamed sequentially).  Layer-2 gathers rows of u2 = h1 @ W11 from an
all-gathered HBM table with the Q7 dma_gather extended instruction (int16
indices => edges are pre-split by src < 32768 so each gather uses one base).
Algebraic folds: W0e = We@W00 and Wf = We@W01 fold the embed matmul away;
u2 = h1@W11 folds the conv1 W1 matmul into the gather table (computed
node-major directly as h1T_g^T @ W11, no transpose pass); embed bias enters
as a rank-1 (v0 x sw) matmul and a constant bias vector.
"""

import sys

sys.path.insert(0, "/opt/trn_rl_repo")

import numpy as np
import ml_dtypes

B, E, D = 32, 2048, 64
EMB, HID, PRED, NPRED = 128, 64, 3, 12
N = B * E
NCORES = 8
NPC = N // NCORES          # 8192 nodes/core
NGRP = NPC // 128          # 64 dst groups of 128 per core
HALF = N // 2              # int16 index limit -> split table in two halves
GB = 4                     # dst-groups per dma_gather batch

_CACHE = {}


def _prep(src, dst, wgt, x):
    """Route edges to (core, group, src-half), chunk into 128-edge chunks.

    Chunk counts per (group, half) are maxed over cores so all cores share one
    program.  Returns compile-time chunk table + per-core device arrays.
    """
    core = dst >> 13
    grp = (dst & (NPC - 1)) >> 7
    half = (src >= HALF).astype(np.int64)
    key = ((core * NGRP + grp) * 2 + half)
    order = np.argsort(key, kind="stable")
    src_s, dst_s, wgt_s = src[order], dst[order], wgt[order]
    bounds = np.searchsorted(key[order], np.arange(NCORES * NGRP * 2 + 1))
    cnt = np.diff(bounds).reshape(NCORES, NGRP, 2)
    # chunks per (group, half): shared across cores
    nch = np.ceil(cnt.max(axis=0) / 128).astype(np.int64)        # [NGRP, 2]
    ncht = int(nch.sum())
    ch_start = np.zeros((NGRP, 2), np.int64)                     # chunk offset
    acc = 0
    for g in range(NGRP):
        for h in range(2):
            ch_start[g, h] = acc
            acc += nch[g, h]

    xg = np.zeros((NCORES, 128, ncht, D), ml_dtypes.bfloat16)
    dst_t = np.zeros((NCORES, 128, ncht), np.float32)
    wgt_t = np.zeros((NCORES, 128, ncht), np.float32)
    idx_t = np.zeros((NCORES, 128, ncht * 8), np.int16)          # 128 idx/chunk -> 8 cols
    for c in range(NCORES):
        for g in range(NGRP):
            for h in range(2):
                lo = bounds[(c * NGRP + g) * 2 + h]
                hi = bounds[(c * NGRP + g) * 2 + h + 1]
                n = hi - lo
                if n == 0:
                    continue
                s_e = src_s[lo:hi]
                d_e = dst_s[lo:hi] & 127
                w_e = wgt_s[lo:hi]
                q0 = ch_start[g, h]
                sl = np.arange(n)
                ch, pt = q0 + sl // 128, sl % 128
                xg[c, pt, ch] = x[s_e]
                dst_t[c, pt, ch] = d_e.astype(np.float32)
                wgt_t[c, pt, ch] = w_e
                # idx layout: chunk q slots j -> flat j16 = q*128 + j;
                # tile[p, f] = idxflat[f*16 + p%16]
                flat = q0 * 128 + sl
                fcol, prow = flat // 16, flat % 16
                v = (s_e - h * HALF).astype(np.int16)
                for rep in range(8):
                    idx_t[c, prow + rep * 16, fcol] = v
    return nch, ch_start, ncht, xg, dst_t, wgt_t, idx_t


def _build_nc(nch, ch_start, ncht, n_cores, phase=3):
    import os
    import concourse.bacc as bacc
    import concourse.bass as bass
    import concourse.mybir as mybir
    import concourse.tile as tile

    f32 = mybir.dt.float32
    bf16 = mybir.dt.bfloat16
    i16 = mybir.dt.int16
    AF = mybir.ActivationFunctionType
    OP = mybir.AluOpType

    nc = bacc.Bacc("TRN2", target_bir_lowering=False, num_devices=n_cores,
                   num_swdge_queues=4)

    # inputs
    t_xT = nc.dram_tensor("xT", [D, NPC], bf16, kind="ExternalInput")
    t_xg = nc.dram_tensor("xg", [128, ncht, D], bf16, kind="ExternalInput")
    t_dst = nc.dram_tensor("dstc", [128, ncht], f32, kind="ExternalInput")
    t_wgt = nc.dram_tensor("wgtc", [128, ncht], f32, kind="ExternalInput")
    t_idx = nc.dram_tensor("idxc", [128, ncht * 8], i16, kind="ExternalInput")
    t_sw = nc.dram_tensor("sw", [1, NPC], bf16, kind="ExternalInput")
    t_W0e = nc.dram_tensor("W0e", [D, EMB], bf16, kind="ExternalInput")
    t_Wf = nc.dram_tensor("Wf", [D, EMB], bf16, kind="ExternalInput")
    t_v0 = nc.dram_tensor("v0", [1, EMB], bf16, kind="ExternalInput")
    t_W10 = nc.dram_tensor("W10", [EMB, EMB], bf16, kind="ExternalInput")
    t_W11 = nc.dram_tensor("W11", [EMB, EMB], bf16, kind="ExternalInput")
    t_bb0 = nc.dram_tensor("bb0", [EMB, 1], f32, kind="ExternalInput")
    t_b1 = nc.dram_tensor("b1", [EMB, 1], f32, kind="ExternalInput")
    t_mW1 = nc.dram_tensor("mW1", [EMB, HID], bf16, kind="ExternalInput")
    t_mb1 = nc.dram_tensor("mb1", [HID, 1], f32, kind="ExternalInput")
    t_mW2 = nc.dram_tensor("mW2", [HID, PRED], bf16, kind="ExternalInput")
    t_mb2 = nc.dram_tensor("mb2", [PRED, 1], f32, kind="ExternalInput")
    t_iota = nc.dram_tensor("iota", [128, 128], bf16, kind="ExternalInput")
    t_out = nc.dram_tensor("o", [PRED, B // n_cores], f32, kind="ExternalOutput")

    # internal DRAM
    t_ush = nc.dram_tensor("ushard", [NPC, EMB], bf16, kind="Internal")
    t_ufull = nc.dram_tensor("ufull", [n_cores * NPC, EMB], bf16,
                             kind="Internal", addr_space="Shared")

    BPC = B // n_cores
    EPC = NPC // BPC  # nodes per batch on this core (2048)

    with tile.TileContext(nc) as tc:
        with (
            tc.tile_pool(name="const", bufs=1) as cpool,
            tc.tile_pool(name="xg", bufs=4) as xgpool,
            tc.tile_pool(name="oh", bufs=6) as ohpool,
            tc.tile_pool(name="gat", bufs=3) as gatpool,
            tc.tile_pool(name="ev", bufs=4) as evpool,
            tc.tile_pool(name="psA", bufs=3, space=bass.MemorySpace.PSUM) as psApool,
            tc.tile_pool(name="ps1", bufs=2, space=bass.MemorySpace.PSUM) as ps1pool,
            tc.tile_pool(name="psU", bufs=2, space=bass.MemorySpace.PSUM) as psUpool,
            tc.tile_pool(name="psM", bufs=1, space=bass.MemorySpace.PSUM) as psMpool,
        ):
            # persistent SBUF
            xT = cpool.tile([D, NPC], bf16)
            dstc = cpool.tile([128, ncht], f32)
            wgtc = cpool.tile([128, ncht], f32)
            idxc = cpool.tile([128, ncht * 8], i16)
            sw = cpool.tile([1, NPC], bf16)
            W0e = cpool.tile([D, EMB], bf16)
            Wf = cpool.tile([D, EMB], bf16)
            v0 = cpool.tile([1, EMB], bf16)
            W10 = cpool.tile([EMB, EMB], bf16)
            W11 = cpool.tile([EMB, EMB], bf16)
            bb0 = cpool.tile([EMB, 1], f32)
            b1 = cpool.tile([EMB, 1], f32)
            mW1 = cpool.tile([EMB, HID], bf16)
            mb1 = cpool.tile([HID, 1], f32)
            mW2 = cpool.tile([HID, PRED], bf16)
            mb2 = cpool.tile([PRED, 1], f32)
            iota = cpool.tile([128, 128], bf16)
            h1T = cpool.tile([EMB, NPC], bf16)
            h2T = cpool.tile([EMB, NPC], bf16)

            for t_src, t_dst_ in [
                (t_xT, xT), (t_dst, dstc), (t_wgt, wgtc), (t_idx, idxc),
                (t_sw, sw), (t_W0e, W0e), (t_Wf, Wf), (t_v0, v0),
                (t_W10, W10), (t_W11, W11), (t_bb0, bb0), (t_b1, b1),
                (t_mW1, mW1), (t_mb1, mb1), (t_mW2, mW2), (t_mb2, mb2),
                (t_iota, iota),
            ]:
                nc.sync.dma_start(t_dst_[:], t_src[:])

            # ---------------- layer 1 ----------------
            for g in range(NGRP):
                gsl = slice(g * 128, (g + 1) * 128)
                q0 = int(ch_start[g, 0])
                nchg = int(nch[g, 0] + nch[g, 1])   # contiguous chunk range
                xg_t = xgpool.tile([128, nchg, D], bf16, tag="xg")
                nc.sync.dma_start(xg_t[:], t_xg[:, q0:q0 + nchg, :])
                psA = psApool.tile([D, 128], f32)
                for i in range(nchg):
                    q = q0 + i
                    oh = ohpool.tile([128, 128], bf16, tag="oh")
                    nc.vector.tensor_scalar(
                        oh[:], iota[:], dstc[:, q:q + 1], wgtc[:, q:q + 1],
                        OP.is_equal, OP.mult)
                    nc.tensor.matmul(psA[:], xg_t[:, i, :], oh[:],
                                     start=(i == 0), stop=(i == nchg - 1))
                A_s = evpool.tile([D, 128], bf16, tag="As")
                nc.vector.tensor_copy(A_s[:], psA[:])
                ps1 = ps1pool.tile([EMB, 128], f32)
                nc.tensor.matmul(ps1[:], W0e[:], xT[:, gsl],
                                 start=True, stop=False)
                nc.tensor.matmul(ps1[:], v0[:], sw[:, gsl],
                                 start=False, stop=False)
                nc.tensor.matmul(ps1[:], Wf[:], A_s[:], start=False, stop=True)
                nc.scalar.activation(h1T[:, gsl], ps1[:], AF.Relu, bias=bb0[:])
                # u2 shard, node-major: u2_g = h1T_g^T @ W11
                psU = psUpool.tile([128, EMB], f32)
                nc.tensor.matmul(psU[:], h1T[:, gsl], W11[:],
                                 start=True, stop=True)
                u_s = evpool.tile([128, EMB], bf16, tag="us")
                nc.scalar.activation(u_s[:], psU[:], AF.Copy)
                nc.sync.dma_start(t_ush[gsl, :], u_s[:])

            # ---------------- all-gather ----------------
            if phase >= 2:
                nc.gpsimd.collective_compute(
                    "AllGather", OP.bypass,
                    replica_groups=[list(range(n_cores))],
                    ins=[t_ush.ap()], outs=[t_ufull.ap()])

            # ---------------- layer 2 ----------------
            qrr = [0]
            for gb in range(NGRP // GB):
                for g in range(gb * GB, (gb + 1) * GB):
                    gsl = slice(g * 128, (g + 1) * 128)
                    nch0, nch1 = int(nch[g, 0]), int(nch[g, 1])
                    nchg = nch0 + nch1
                    q0 = int(ch_start[g, 0])
                    gt = gatpool.tile([128, nchg, EMB], bf16, tag="gt")
                    if phase >= 3:
                        for h, (qh, nh) in ((0, (q0, nch0)), (1, (q0 + nch0, nch1))):
                            for p0 in range(0, nh, 8):
                                pn = min(8, nh - p0)
                                nidx = pn * 128
                                col0 = (qh + p0) * 8
                                nc.gpsimd.dma_gather(
                                    gt[:, qh - q0 + p0:qh - q0 + p0 + pn, :],
                                    t_ufull[h * HALF:(h + 1) * HALF, :],
                                    idxc[:, col0:col0 + nidx // 16],
                                    nidx, nidx, EMB,
                                    queue_num=qrr[0] % 4)
                                qrr[0] += 1
                    else:
                        nc.vector.memset(gt[:], 0.0)
                    ps1 = ps1pool.tile([EMB, 128], f32)
                    nc.tensor.matmul(ps1[:], W10[:], h1T[:, gsl],
                                     start=True, stop=False)
                    for i in range(nchg):
                        q = q0 + i
                        oh = ohpool.tile([128, 128], bf16, tag="oh")
                        nc.vector.tensor_scalar(
                            oh[:], iota[:], dstc[:, q:q + 1], wgtc[:, q:q + 1],
                            OP.is_equal, OP.mult)
                        nc.tensor.matmul(ps1[:], gt[:, i, :], oh[:],
                                         start=False, stop=(i == nchg - 1))
                    nc.scalar.activation(h2T[:, gsl], ps1[:], AF.Relu, bias=b1[:])

            # ---------------- pool + MLP ----------------
            hm_f = evpool.tile([EMB, BPC], f32, tag="hmf")
            nc.vector.tensor_reduce(
                out=hm_f[:],
                in_=h2T[:].rearrange("p (b e) -> p b e", b=BPC),
                op=OP.add, axis=mybir.AxisListType.X)
            hm = evpool.tile([EMB, BPC], bf16, tag="hm")
            nc.vector.tensor_copy(hm[:], hm_f[:])
            psM = psMpool.tile([HID, BPC], f32)
            nc.tensor.matmul(psM[:], mW1[:], hm[:], start=True, stop=True)
            z = evpool.tile([HID, BPC], bf16, tag="z")
            nc.scalar.activation(z[:], psM[:], AF.Relu, bias=mb1[:])
            psO = psMpool.tile([PRED, BPC], f32, tag="psM")
            nc.tensor.matmul(psO[:], mW2[:], z[:], start=True, stop=True)
            o_s = evpool.tile([PRED, BPC], f32, tag="os")
            nc.vector.tensor_scalar(o_s[:], psO[:], mb2[:], None, OP.add)
            nc.sync.dma_start(t_out[:], o_s[:])

    nc.compile()
    return nc


def kernel(**inputs):
    x = np.asarray(inputs["x"], np.float32).reshape(N, D)
    edge_index = np.asarray(inputs["edge_index"])
    We = np.asarray(inputs["embed_W"], np.float32)
    be = np.asarray(inputs["embed_b"], np.float32)
    W00 = np.asarray(inputs["conv0_W0"], np.float32)
    W01 = np.asarray(inputs["conv0_W1"], np.float32)
    b0 = np.asarray(inputs["conv0_b"], np.float32)
    W10 = np.asarray(inputs["conv1_W0"], np.float32)
    W11 = np.asarray(inputs["conv1_W1"], np.float32)
    b1 = np.asarray(inputs["conv1_b"], np.float32)
    mW1 = np.asarray(inputs["mlp_W1"], np.float32)
    mb1 = np.asarray(inputs["mlp_b1"], np.float32)
    mW2 = np.asarray(inputs["mlp_W2"], np.float32)
    mb2 = np.asarray(inputs["mlp_b2"], np.float32)

    src = np.asarray(edge_index[0]).astype(np.int64)
    dst = np.asarray(edge_index[1]).astype(np.int64)
    deg = np.bincount(src, minlength=N).astype(np.float32)
    dinv = np.where(deg > 0, 1.0 / np.sqrt(np.maximum(deg, 1e-12)), 0.0)
    wgt = (-dinv[src] * dinv[dst]).astype(np.float32)

    nch, ch_start, ncht, xg, dst_t, wgt_t, idx_t = _prep(src, dst, wgt, x)

    sw_full = np.zeros(N, np.float32)
    np.add.at(sw_full, dst, wgt)

    bf = ml_dtypes.bfloat16
    xT_sh = x.reshape(NCORES, NPC, D).transpose(0, 2, 1).astype(bf)
    consts = {
        "sw": sw_full.reshape(NCORES, 1, NPC).astype(bf),
        "W0e": (We @ W00).astype(bf),
        "Wf": (We @ W01).astype(bf),
        "v0": (W01.T @ be).reshape(1, EMB).astype(bf),
        "W10": W10.astype(bf),
        "W11": W11.astype(bf),
        "bb0": (b0 + W00.T @ be).reshape(EMB, 1).astype(np.float32),
        "b1": b1.reshape(EMB, 1).astype(np.float32),
        "mW1": (mW1 / E).astype(bf),
        "mb1": mb1.reshape(HID, 1).astype(np.float32),
        "mW2": mW2.astype(bf),
        "mb2": mb2.reshape(PRED, 1).astype(np.float32),
        "iota": np.tile(np.arange(128, dtype=np.float32), (128, 1)).astype(bf),
    }

    key = (tuple(nch.ravel()), ncht)
    if _CACHE.get("key") != key:
        _CACHE["nc"] = _build_nc(nch, ch_start, ncht, NCORES)
        _CACHE["key"] = key
    nc = _CACHE["nc"]

    from concourse.bass_utils import run_bass_kernel_spmd

    in_maps = []
    for c in range(NCORES):
        m = {
            "xT": np.ascontiguousarray(xT_sh[c]),
            "xg": np.ascontiguousarray(xg[c]),
            "dstc": np.ascontiguousarray(dst_t[c]),
            "wgtc": np.ascontiguousarray(wgt_t[c]),
            "idxc": np.ascontiguousarray(idx_t[c]),
            "sw": np.ascontiguousarray(consts["sw"][c]),
        }
        for k in ("W0e", "Wf", "v0", "W10", "W11", "bb0", "b1",
                  "mW1", "mb1", "mW2", "mb2", "iota"):
            m[k] = consts[k]
        in_maps.append(m)

    res = run_bass_kernel_spmd(nc, in_maps, list(range(NCORES)))
    out1 = np.zeros((B, PRED), np.float32)
    for c in range(NCORES):
        o = np.asarray(res.results[c]["o"], np.float32)   # [PRED, BPC]
        out1[c * (B // NCORES):(c + 1) * (B // NCORES), :] = o.T
    full = np.broadcast_to(out1[:, None, None, :], (B, NPRED, E, PRED))
    return np.ascontiguousarray(full, dtype=np.float32)


kernel._jit_holder = _CACHE
